# revision 15
# baseline (speedup 1.0000x reference)
"""MoE expert FFN (grouped GEMM) Trainium2 kernel.

Problem: inputs [W=8, E=4, C=2048, H=1024] fp32, per-expert FFN
(W1 [E,H,4F], b1, W2 [E,4F,H], b2) with tanh-approx GELU between.
out[w,e,c,:] = FFN_e(inputs[w,e,c,:]).

Sharding (expert-parallel x token-parallel, 8 cores): core c handles
expert e = c//2 and world-slice w in [0,4) or [4,8) by c%2 -> 8192
tokens per core, one expert's weights per core.

Device layout: everything is pre-transposed on the host so the
contraction dim always lands on SBUF partitions and no on-chip
transposes are needed:
  xt  [128, 8, T]    bf16   xt[p,k,t]  = X[t, k*128+p]     (X = tokens [T,1024])
  w1  [128, 8, 4096] bf16   w1[p,k,f]  = W1[k*128+p, f]
  w2  [128, 32,1024] bf16   w2[p,k,f]  = W2[k*128+p, f]
  b1  [128, 32]      f32    b1[p,m]    = b1_full[m*128+p]
  b2  [128, 8]       f32    b2[p,m]    = b2_full[m*128+p]
  out [128, 8, T]    f32    out[p,m,t] = Y[t, m*128+p]

Per 512-token chunk: GEMM1 accumulates 8 k-tiles into a PSUM bank per
dff-tile (32 of them), ACT applies bias+gelu PSUM->SBUF bf16, GEMM2
accumulates 32 k-tiles per h-tile (8), DVE adds b2 PSUM->SBUF f32,
DMA out. Both weight matrices stay SBUF-resident (128 KiB/partition).

The matmul phase runs at the bf16 streaming bound (N=512 cols @ 1
col/cycle @ 2.4GHz + ~2.5ns NX issue = ~216ns per matmul, 8192
matmuls/core). Startup is DMA-bound: the critical x-chunk-0 + first w1
tiles are split across both HWDGE queues and the remaining weight /
prefetch DMAs carry scheduler wait-timestamps so they don't dilute
early DMA bandwidth (service is round-robin over inflight descriptors).
The final output group is split N=128 so the post-matmul drain is
short. fp8/DoubleRow was evaluated and rejected: e4m3 quantization of
both operands gives rel-L2 ~5e-2 vs the 2e-2 gate, and hi/lo
decompositions burn the 2x rate advantage.
"""

import sys
from contextlib import ExitStack

import numpy as np

for _p in ("/opt/trn_rl_repo",):
    if _p not in sys.path:
        sys.path.insert(0, _p)

import ml_dtypes

import concourse.bacc as bacc
import concourse.tile as tile
from concourse import mybir
from concourse.bass_utils import run_bass_kernel_spmd

BF16 = ml_dtypes.bfloat16

W, E, C, H = 8, 4, 2048, 1024
DFF = 4 * H
N_CORES = 8
P = 128
T = (W // 2) * C          # tokens per core = 8192
KH = H // P               # 8 k-tiles over H
KF = DFF // P             # 32 k-tiles over DFF
NCHUNK = 512
NT = T // NCHUNK          # 16 chunks

_PROG = None              # cached compiled program


def build_program():
    nc = bacc.Bacc("TRN2", target_bir_lowering=False, debug=False,
                   num_devices=N_CORES)
    xt_ap = nc.dram_tensor("xt", [P, KH, T], mybir.dt.bfloat16,
                           kind="ExternalInput").ap()
    # weights grouped by OUTPUT tile m (all k-slices of one m are one
    # contiguous DMA), so each m-tile's matmuls unblock independently
    w1_ap = nc.dram_tensor("w1", [P, KF, KH, P], mybir.dt.bfloat16,
                           kind="ExternalInput").ap()
    w2_ap = nc.dram_tensor("w2", [P, KH, KF, P], mybir.dt.bfloat16,
                           kind="ExternalInput").ap()
    b1_ap = nc.dram_tensor("b1", [P, KF], mybir.dt.float32,
                           kind="ExternalInput").ap()
    b2_ap = nc.dram_tensor("b2", [P, KH], mybir.dt.float32,
                           kind="ExternalInput").ap()
    out_ap = nc.dram_tensor("out", [P, KH, T], mybir.dt.float32,
                            kind="ExternalOutput").ap()

    gelu = mybir.ActivationFunctionType.Gelu_apprx_tanh

    with tile.TileContext(nc) as tc:
        with ExitStack() as ctx:
            wpool = ctx.enter_context(tc.tile_pool(name="weights", bufs=1))
            xpool = ctx.enter_context(tc.tile_pool(name="x", bufs=2))
            gpool = ctx.enter_context(tc.tile_pool(name="g", bufs=1))
            opool = ctx.enter_context(tc.tile_pool(name="o", bufs=4))
            ps1 = ctx.enter_context(tc.tile_pool(name="ps1", bufs=4,
                                                 space="PSUM"))
            ps2 = ctx.enter_context(tc.tile_pool(name="ps2", bufs=4,
                                                 space="PSUM"))

            w1_sb = wpool.tile([P, KF, KH, P], mybir.dt.bfloat16, tag="w1")
            w2_sb = wpool.tile([P, KH, KF, P], mybir.dt.bfloat16, tag="w2")
            b1_sb = wpool.tile([P, KF], mybir.dt.float32, tag="b1")
            b2_sb = wpool.tile([P, KH], mybir.dt.float32, tag="b2")
            # Short PE warmup (no DMA deps): runs ~8-10us while the
            # first transfers land, starting the HAM activity window so
            # the clock gate opens during the DMA-paced first m-group.
            warm_sb = wpool.tile([P, 256], mybir.dt.bfloat16, tag="warm")
            nc.vector.memset(warm_sb[:], 0)
            warm_ps = ps1.tile([P, 256], mybir.dt.float32, tag="ps1",
                               name="warm_ps")
            for i in range(7):
                nc.tensor.matmul(warm_ps[:], lhsT=warm_sb[:, :P],
                                 rhs=warm_sb[:], start=(i == 0),
                                 stop=(i == 6))

            # One DMA descriptor is serviced by ONE dma engine (~130GB/s);
            # parallelism comes only from having several descriptors in
            # flight, so x chunk 0 stays split per k-tile. The critical
            # first ~2MB (x0 + w1 m0-m2 + biases) is split across BOTH
            # HWDGE queues (descriptor issue is ~650ns each, strictly
            # serial per queue). Only 6 descriptors go on the scalar
            # queue, all done well before the first ACT exists; the bulk
            # of the weights stays on sync so weight issues never block
            # ACTs (and never hit the 16-deep queue-full stall).
            x_tiles = {}
            x_tiles[0] = xpool.tile([P, KH, NCHUNK], mybir.dt.bfloat16,
                                    tag="x", name="x_sb")

            def dma_x0(eng, k):
                eng.dma_start(x_tiles[0][:, k, :], xt_ap[:, k, 0:NCHUNK])

            nc.scalar.dma_start(w1_sb[:, 0], w1_ap[:, 0])
            dma_x0(nc.sync, 0)
            dma_x0(nc.scalar, 1)
            dma_x0(nc.sync, 2)
            dma_x0(nc.scalar, 3)
            dma_x0(nc.sync, 4)
            dma_x0(nc.scalar, 5)
            dma_x0(nc.sync, 6)
            dma_x0(nc.scalar, 7)
            nc.sync.dma_start(w1_sb[:, 1], w1_ap[:, 1])
            nc.scalar.dma_start(w1_sb[:, 2], w1_ap[:, 2])
            # Early DMA service is round-robin across ALL inflight
            # descriptors, so eagerly issuing every weight tile dilutes
            # the bandwidth of the critical x0/w1[m0..m2] set (measured:
            # first matmul slipped 10.6us -> 13.1us). Stage the rest with
            # scheduler wait timestamps, pacing slightly faster than the
            # ~1.73us/tile consumption so they stay just ahead.
            with tc.tile_wait_until(0.005):
                nc.sync.dma_start(b1_sb[:], b1_ap[:])
            for m in range(3, KF):
                with tc.tile_wait_until(0.003 + 0.0012 * m):
                    nc.sync.dma_start(w1_sb[:, m], w1_ap[:, m])
            with tc.tile_wait_until(0.010):
                nc.sync.dma_start(b2_sb[:], b2_ap[:])
            for m in range(KH):
                with tc.tile_wait_until(0.042 + 0.002 * m):
                    nc.sync.dma_start(w2_sb[:, m], w2_ap[:, m])

            for c in range(NT):
                tok = slice(c * NCHUNK, (c + 1) * NCHUNK)
                if c not in x_tiles:
                    x_tiles[c] = xpool.tile([P, KH, NCHUNK],
                                            mybir.dt.bfloat16,
                                            tag="x", name="x_sb")
                    # waited so chunk prefetches can't get scheduled ahead
                    # of the startup weight DMAs and steal early bandwidth
                    with tc.tile_wait_until(0.04 + 0.105 * (c - 1)):
                        nc.sync.dma_start(x_tiles[c][:], xt_ap[:, :, tok])
                x_sb = x_tiles.pop(c)

                g_sb = gpool.tile([P, KF, NCHUNK], mybir.dt.bfloat16, tag="g")
                for m in range(KF):
                    pt = ps1.tile([P, NCHUNK], mybir.dt.float32, tag="ps1")
                    for k in range(KH):
                        nc.tensor.matmul(
                            pt[:],
                            lhsT=w1_sb[:, m, k, :],
                            rhs=x_sb[:, k, :],
                            start=(k == 0), stop=(k == KH - 1))
                    nc.scalar.activation(g_sb[:, m, :], pt[:], gelu,
                                         bias=b1_sb[:, m:m + 1], scale=1.0)

                for m in range(KH):
                    pt2 = ps2.tile([P, NCHUNK], mybir.dt.float32, tag="ps2")
                    if c == NT - 1 and m == KH - 1:
                        # final m-group: 4 independent N=128 sub-groups so
                        # the last matmul -> DVE -> out-DMA drain is ~64KB
                        # instead of 256KB. Each sub-group gets its own
                        # PSUM tile: slices of one tile create false
                        # whole-tile deps (MM group j+1 waits on DVE j,
                        # ~450ns hiccup each).
                        for j in range(4):
                            ptj = pt2 if j == 0 else ps2.tile(
                                [P, NCHUNK], mybir.dt.float32, tag="ps2")
                            cols = slice(j * 128, (j + 1) * 128)
                            tokj = slice(c * NCHUNK + j * 128,
                                         c * NCHUNK + (j + 1) * 128)
                            for k in range(KF):
                                nc.tensor.matmul(
                                    ptj[:, cols],
                                    lhsT=w2_sb[:, m, k, :],
                                    rhs=g_sb[:, k, cols],
                                    start=(k == 0), stop=(k == KF - 1))
                            o_sb = opool.tile([P, 128], mybir.dt.float32,
                                              tag="otail")
                            nc.vector.tensor_scalar_add(o_sb[:], ptj[:, cols],
                                                        b2_sb[:, m:m + 1])
                            nc.sync.dma_start(out_ap[:, m, tokj], o_sb[:])
                        continue
                    for k in range(KF):
                        nc.tensor.matmul(
                            pt2[:],
                            lhsT=w2_sb[:, m, k, :],
                            rhs=g_sb[:, k, :],
                            start=(k == 0), stop=(k == KF - 1))
                    o_sb = opool.tile([P, NCHUNK], mybir.dt.float32, tag="o")
                    nc.vector.tensor_scalar_add(o_sb[:], pt2[:],
                                                b2_sb[:, m:m + 1])
                    nc.sync.dma_start(out_ap[:, m, tok], o_sb[:])

    nc.compile()
    return nc


def _get_prog():
    global _PROG
    if _PROG is None:
        _PROG = build_program()
    return _PROG


def _shard_inputs(inputs, W1, b1, W2, b2):
    inputs = np.asarray(inputs, dtype=np.float32)
    W1 = np.asarray(W1, dtype=np.float32)
    b1 = np.asarray(b1, dtype=np.float32)
    W2 = np.asarray(W2, dtype=np.float32)
    b2 = np.asarray(b2, dtype=np.float32)
    in_maps = []
    for core in range(N_CORES):
        e = core // 2
        wlo = (core % 2) * (W // 2)
        X = np.ascontiguousarray(inputs[wlo:wlo + W // 2, e]).reshape(T, H)
        Xb = X.astype(BF16)
        # [T,H] -> [H,T] -> [KH,P,T] -> [P,KH,T]
        xt = np.ascontiguousarray(
            Xb.T.reshape(KH, P, T).transpose(1, 0, 2))
        # W1[h,f], h=k*128+p, f=m*128+c -> [p, m, k, c]
        w1 = np.ascontiguousarray(
            W1[e].astype(BF16).reshape(KH, P, KF, P).transpose(1, 2, 0, 3))
        # W2[f,h], f=k*128+p, h=m*128+c -> [p, m, k, c]
        w2 = np.ascontiguousarray(
            W2[e].astype(BF16).reshape(KF, P, KH, P).transpose(1, 2, 0, 3))
        b1c = np.ascontiguousarray(b1[e].reshape(KF, P).T)
        b2c = np.ascontiguousarray(b2[e].reshape(KH, P).T)
        in_maps.append({"xt": xt, "w1": w1, "w2": w2, "b1": b1c, "b2": b2c})
    return in_maps


def _unshard(results):
    out = np.empty((W, E, C, H), dtype=np.float32)
    for core in range(N_CORES):
        e = core // 2
        wlo = (core % 2) * (W // 2)
        dev = results[core]["out"]                      # [P, KH, T]
        Y = dev.transpose(2, 1, 0).reshape(W // 2, C, H)  # [t,m,p] -> [T,H]
        out[wlo:wlo + W // 2, e] = Y
    return out


def run_sharded(in_maps, **kwargs):
    """Compile (cached) + run on cores 0-7; returns BassKernelResults."""
    nc = _get_prog()
    return run_bass_kernel_spmd(nc, in_maps, list(range(N_CORES)), **kwargs)


def kernel(inputs, W1, b1, W2, b2):
    in_maps = _shard_inputs(inputs, W1, b1, W2, b2)
    res = run_sharded(in_maps)
    return _unshard(res.results)

